# revision 6
# baseline (speedup 1.0000x reference)
"""GAT-style sparse neighbor aggregation kernel for Trainium2 (8 NeuronCores).

Reference computation (dense):
    hf = X @ W; he = E @ W
    e  = leakyrelu((hf@a1)[:,None] + (he@a2)[None,:])
    att = softmax(where(mask, e, -9e15), axis=1)     # mask: <=10 nnz/row
    out = att @ he

Key algebra: att @ he == (att @ E) @ W, and att is row-sparse (<=K nnz).
So per row i:  out_i = (sum_k w_ik * E[idx_ik]) @ W   with
    s_ik = leakyrelu(f_i + g_ik),  f = X @ (W@a1),  g_ik = E[idx_ik]. (W@a2)
    w_ik = softmax over the deduplicated k's.
This turns 56 GFLOP dense into ~5 GFLOP total.

Sharding: batch rows N=2048 split across 8 cores (256 rows each); W, a, E
replicated (E only touched via row gathers + one pass for W@a vectors).

Engine mapping per core:
  - gathers: gpsimd indirect DMA, one [128 rows x 4KB] gather per (t, k)
  - dot products (a'=W@a, f=X.a1', g=eg.a2'): DVE tensor_mul + ACT accum-reduce
  - softmax over k: DVE small ops + ACT fused exp/sum
  - aggregation sum_k w*eg AND its transpose: PE fp32 matmuls
    (lhsT=eg chunk, rhs=diag(w)) accumulating aggT directly in PSUM
  - final agg @ W: PE float32r matmuls (full rate, N=512)
"""

import os
import sys

import numpy as np

sys.path.insert(0, "/opt/trn_rl_repo")

from contextlib import ExitStack

import concourse.bass as bass
import concourse.tile as tile
from concourse import bacc, mybir
from concourse.bass_utils import run_bass_kernel_spmd
from concourse.masks import make_identity

N, M, F, K = 2048, 8192, 1024, 10
NCORES = 8
NL = N // NCORES  # 256 rows per core
P = 128
T = NL // P  # row-tiles per core (2)
FC = F // P  # feature chunks (8)
NH = 512  # matmul free-dim half (psum bank)
ALPHA = 0.2
NEGBIG = -1e30

f32 = mybir.dt.float32
f32r = mybir.dt.float32r
i32 = mybir.dt.int32
AX = mybir.AxisListType
OP = mybir.AluOpType
ACT = mybir.ActivationFunctionType

USE_F32R = os.environ.get("KERNEL_F32R", "1") == "1"
MM_DT = f32r if USE_F32R else f32


def build_kernel():
    nc = bacc.Bacc("TRN2", target_bir_lowering=False, debug=False, num_devices=NCORES)

    feat = nc.dram_tensor("feat", [NL, F], f32, kind="ExternalInput").ap()
    w = nc.dram_tensor("w", [F, F], f32, kind="ExternalInput").ap()
    emb = nc.dram_tensor("emb", [M, F], f32, kind="ExternalInput").ap()
    av = nc.dram_tensor("av", [2 * F], f32, kind="ExternalInput").ap()
    idx = nc.dram_tensor("idx", [NL, K], i32, kind="ExternalInput").ap()
    dneg = nc.dram_tensor("dneg", [NL, K], f32, kind="ExternalInput").ap()
    out = nc.dram_tensor("out", [NL, F], f32, kind="ExternalOutput").ap()

    with tile.TileContext(nc) as tc, ExitStack() as ctx:
        const = ctx.enter_context(tc.tile_pool(name="const", bufs=1))
        big = ctx.enter_context(tc.tile_pool(name="big", bufs=1))
        eg_pool = ctx.enter_context(tc.tile_pool(name="eg", bufs=2))
        sm = ctx.enter_context(tc.tile_pool(name="small", bufs=2))
        scr = ctx.enter_context(tc.tile_pool(name="scratch", bufs=4))
        dg = ctx.enter_context(tc.tile_pool(name="diag", bufs=2 * K + 2))
        ps = ctx.enter_context(tc.tile_pool(name="psum", bufs=3, space="PSUM"))
        pso = ctx.enter_context(tc.tile_pool(name="psum_o", bufs=2, space="PSUM"))
        dram = ctx.enter_context(tc.tile_pool(name="dram", bufs=1, space="DRAM"))

        ident = const.tile([P, P], f32)
        make_identity(nc, ident[:])

        # W resident in SBUF as float32r (gpsimd DMA casts): w_sb[p, c, j] = W[c*128+p, j]
        w_sb = big.tile([P, FC, F], MM_DT)
        nc.gpsimd.dma_start(w_sb[:], w.rearrange("(c p) j -> p c j", p=P))
        w_f = w_sb[:].bitcast(f32)

        # raw a1/a2 broadcast to all partitions
        a1b = big.tile([P, F], f32)
        a2b = big.tile([P, F], f32)
        nc.sync.dma_start(a1b[:], av[0:F].unsqueeze(0).partition_broadcast(P))
        nc.sync.dma_start(a2b[:], av[F : 2 * F].unsqueeze(0).partition_broadcast(P))

        # feature rows for this core: feat_sb[p, t, f] = X[t*128+p, f]
        feat_sb = big.tile([P, T, F], f32)
        nc.sync.dma_start(feat_sb[:], feat.rearrange("(t p) f -> p t f", p=P))

        def dot(in0, in1, acc_slice):
            """acc_slice[p, 0] = sum_j in0[p, j] * in1[p, j] (DVE mult + ACT reduce)."""
            m = scr.tile([P, F], f32, tag="mul")
            nc.vector.tensor_mul(out=m[:], in0=in0, in1=in1)
            dmy = sm.tile([P, 1], f32, tag="dummy")
            nc.scalar.activation(
                out=dmy[:].broadcast_to(m[:].shape), in_=m[:],
                func=ACT.Identity, bias=0.0, scale=1.0, accum_out=acc_slice,
            )

        # ---- a1' = W @ a1, a2' = W @ a2 (column layout, then bounce) ----
        abcol = sm.tile([P, 2 * FC], f32, tag="abcol")
        for c in range(FC):
            dot(w_f[:, c, :], a1b[:], abcol[:, c : c + 1])
            dot(w_f[:, c, :], a2b[:], abcol[:, FC + c : FC + c + 1])
        # bounce through DRAM to flatten column layout -> row vectors
        ab_dram = dram.tile([2 * FC, P], f32)
        nc.sync.dma_start(ab_dram[:].rearrange("c p -> p c"), abcol[:])
        ab_flat = ab_dram[:].rearrange("c p -> (c p)")
        a1pb = big.tile([P, F], f32)
        a2pb = big.tile([P, F], f32)
        nc.sync.dma_start(a1pb[:], ab_flat[0:F].unsqueeze(0).partition_broadcast(P))
        nc.sync.dma_start(a2pb[:], ab_flat[F : 2 * F].unsqueeze(0).partition_broadcast(P))

        aggT = big.tile([P, T, FC, P], MM_DT)

        for t in range(T):
            r0 = t * P
            idx_t = sm.tile([P, K], i32, tag="idx")
            nc.sync.dma_start(idx_t[:], idx[r0 : r0 + P, :])
            dn_t = sm.tile([P, K], f32, tag="dn")
            nc.sync.dma_start(dn_t[:], dneg[r0 : r0 + P, :])

            # gather embed rows: eg[p, k, :] = E[idx[r0+p, k], :]
            eg = eg_pool.tile([P, K, F], f32, tag="eg")
            for k in range(K):
                nc.gpsimd.indirect_dma_start(
                    out=eg[:, k, :],
                    out_offset=None,
                    in_=emb,
                    in_offset=bass.IndirectOffsetOnAxis(ap=idx_t[:, k : k + 1], axis=0),
                )

            # g_ik = eg[i,k,:] . a2' ; f_i = X[i,:] . a1'
            g_t = sm.tile([P, K], f32, tag="g")
            for k in range(K):
                dot(eg[:, k, :], a2pb[:], g_t[:, k : k + 1])
            f_t = sm.tile([P, 1], f32, tag="f")
            dot(feat_sb[:, t, :], a1pb[:], f_t[:])

            # scores: s = leakyrelu(g + f) + dup_mask_neg
            s_t = sm.tile([P, K], f32, tag="s")
            nc.vector.tensor_scalar_add(out=s_t[:], in0=g_t[:], scalar1=f_t[:])
            lr = sm.tile([P, K], f32, tag="lr")
            nc.vector.tensor_scalar_mul(out=lr[:], in0=s_t[:], scalar1=ALPHA)
            nc.vector.tensor_tensor(out=s_t[:], in0=s_t[:], in1=lr[:], op=OP.max)
            nc.vector.tensor_tensor(out=s_t[:], in0=s_t[:], in1=dn_t[:], op=OP.add)

            # masked softmax over k (exp and normalizer fused on ACT)
            mx = sm.tile([P, 1], f32, tag="mx")
            nc.vector.tensor_reduce(out=mx[:], in_=s_t[:], axis=AX.X, op=OP.max)
            nmx = sm.tile([P, 1], f32, tag="nmx")
            nc.vector.tensor_scalar_mul(out=nmx[:], in0=mx[:], scalar1=-1.0)
            p_t = sm.tile([P, K], f32, tag="p")
            z_t = sm.tile([P, 1], f32, tag="z")
            nc.scalar.activation(
                out=p_t[:], in_=s_t[:], func=ACT.Exp, bias=nmx[:], scale=1.0,
                accum_out=z_t[:],
            )
            zi = sm.tile([P, 1], f32, tag="zi")
            nc.vector.reciprocal(out=zi[:], in_=z_t[:])
            wts = sm.tile([P, K], f32, tag="wts")
            nc.vector.tensor_scalar_mul(out=wts[:], in0=p_t[:], scalar1=zi[:])

            # diag(w) tiles
            dks = []
            for k in range(K):
                dk = dg.tile([P, P], f32, tag="dk")
                nc.vector.tensor_scalar_mul(out=dk[:], in0=ident[:], scalar1=wts[:, k : k + 1])
                dks.append(dk)

            # aggregation, transposed directly:
            #   aggT[m, n] = sum_k (eg[:, k, c*128+m]).T @ diag(w_k) = w_n * E[idx[n,k], c*128+m]
            for c in range(FC):
                at_ps = ps.tile([P, P], f32, tag="at_ps")
                for k in range(K):
                    nc.tensor.matmul(
                        out=at_ps[:],
                        lhsT=eg[:, k, c * P : (c + 1) * P],
                        rhs=dks[k][:],
                        start=(k == 0),
                        stop=(k == K - 1),
                    )
                nc.vector.tensor_copy(out=aggT[:, t, c, :], in_=at_ps[:])

            # out = agg @ W: out[r, j] = sum_c aggT[:, t, c, r] . W-chunk
            for nh in range(F // NH):
                o_ps = pso.tile([P, NH], f32, tag="o_ps")
                for c in range(FC):
                    nc.tensor.matmul(
                        out=o_ps[:],
                        lhsT=aggT[:, t, c, :],
                        rhs=w_sb[:, c, nh * NH : (nh + 1) * NH],
                        start=(c == 0),
                        stop=(c == FC - 1),
                    )
                ob = scr.tile([P, NH], f32, tag="ob")
                nc.vector.tensor_copy(out=ob[:], in_=o_ps[:])
                nc.sync.dma_start(out[r0 : r0 + P, nh * NH : (nh + 1) * NH], ob[:])

    nc.compile()
    return nc


_NC_CACHE = None


def _get_nc():
    global _NC_CACHE
    if _NC_CACHE is None:
        _NC_CACHE = build_kernel()
    return _NC_CACHE


def _host_prep(feature_matrix, embed_matrix, weight, a, neigh_idx):
    feature_matrix = np.ascontiguousarray(np.asarray(feature_matrix, dtype=np.float32))
    embed_matrix = np.ascontiguousarray(np.asarray(embed_matrix, dtype=np.float32))
    weight = np.ascontiguousarray(np.asarray(weight, dtype=np.float32))
    av = np.ascontiguousarray(np.asarray(a, dtype=np.float32).reshape(2 * F))
    idx = np.asarray(neigh_idx)
    idx32 = np.ascontiguousarray(idx.astype(np.int32))

    # duplicate-index mask (set semantics): only first occurrence is valid
    dup = np.zeros((N, K), dtype=bool)
    for k in range(1, K):
        dup[:, k] = (idx[:, :k] == idx[:, k : k + 1]).any(axis=1)
    dneg = np.where(dup, np.float32(NEGBIG), np.float32(0.0)).astype(np.float32)

    in_maps = []
    for c in range(NCORES):
        sl = slice(c * NL, (c + 1) * NL)
        in_maps.append(
            {
                "feat": feature_matrix[sl],
                "w": weight,
                "emb": embed_matrix,
                "av": av,
                "idx": idx32[sl],
                "dneg": dneg[sl],
            }
        )
    return in_maps


def run(inputs, trace=False, **kw):
    nc = _get_nc()
    in_maps = _host_prep(**inputs)
    res = run_bass_kernel_spmd(nc, in_maps, core_ids=list(range(NCORES)), trace=trace, **kw)
    out = np.concatenate([res.results[c]["out"] for c in range(NCORES)], axis=0)
    return out, res


def kernel(**inputs) -> np.ndarray:
    out, _ = run(inputs, trace=False)
    return out
